# revision 4
# baseline (speedup 1.0000x reference)
"""Trainium2 Bass kernel for nn_ContextClassifier.

Strategy (8 NeuronCores, SPMD, no collectives):
  - Host gathers span features (forwards/backwards rows), builds transposed
    bf16 FFN inputs, and row-shards the [2N, D] feature matrix: cores 0-3 get
    the 2048 "context" rows, cores 4-7 the 2048 "phrase" rows (512 rows each).
  - Each core: FFN matmul (K-blocked, PSUM-accumulated) -> tanh -> featsT in
    SBUF (bf16), then streams the full W_lab^T [D, V] through SBUF computing
    logits tiles in PSUM and exp-accumulating them on the Scalar engine to get
    per-row sum(exp(logits)) over the whole vocab. Row sharding makes the
    softmax normalizer core-local (no all-reduce needed).
  - Host finishes the tiny tail: tag logit (rowwise dot of feats with
    W_lab[tags]), log-sum-exp, focal loss reduction.
"""

import numpy as np
import ml_dtypes

S, B, H = 512, 32, 512
N = 2048
D = 256
LMAX, LDIM = 16, 32
V = 50257
GAMMA = 2.0
NCORES = 8
SPANS = 2 * N // NCORES        # 512 rows per core
RT = SPANS // 128              # 4 row tiles
KPAD = 2176                    # padded FFN contraction (phr K=2080 -> 17*128)
KB = KPAD // 128               # 17 K blocks
CT = 512                       # vocab column tile
BIGCOLS = 2048                 # psum tile width (4 banks)

BF16 = ml_dtypes.bfloat16

_CACHE = {}


def _vocab_tiling():
    col_tiles = [CT] * (V // CT)
    if V % CT:
        col_tiles.append(V % CT)
    big_tiles = []  # (c0, [widths]) with sum(widths) <= BIGCOLS
    cur, cur0, tot, off = [], 0, 0, 0
    for w in col_tiles:
        if tot + w > BIGCOLS:
            big_tiles.append((cur0, cur))
            cur, cur0, tot = [], off, 0
        cur.append(w)
        tot += w
        off += w
    if cur:
        big_tiles.append((cur0, cur))
    return big_tiles


def _split_multi_waits(nc, mybir, max_waits=1):
    # This walrus build rejects >1 sync wait per instruction; hoist extras
    # onto dedicated EventSemaphore instructions placed just before.
    ctr = 0
    for fn in nc.m.functions:
        for bb in fn.blocks:
            out = []
            for ins in bb.instructions:
                si = ins.sync_info
                if si is not None and si.on_wait and len(si.on_wait) > max_waits:
                    waits = list(si.on_wait)
                    for w in waits[max_waits:]:
                        ev = mybir.InstEventSemaphore(
                            name=f"splitwait_{ctr}", ins=[], outs=[])
                        ctr += 1
                        ev.sync_info = mybir.SyncInfo(on_wait=[w], on_update=[])
                        ev.engine = ins.engine
                        out.append(ev)
                    ins.sync_info = mybir.SyncInfo(
                        on_wait=waits[:max_waits], on_update=list(si.on_update))
                out.append(ins)
            bb.instructions = out
    return ctr


def _build_program(split_waits=True):
    import concourse.bass as bass
    import concourse.mybir as mybir
    import concourse.tile as tile
    from contextlib import ExitStack

    dt = mybir.dt
    AF = mybir.ActivationFunctionType
    big_tiles = _vocab_tiling()
    NBT = len(big_tiles)

    nc = bass.Bass()
    xT_d = nc.dram_tensor("xT", [128, KB, SPANS], dt.bfloat16, kind="ExternalInput")
    wT_d = nc.dram_tensor("wT", [128, KB, D], dt.bfloat16, kind="ExternalInput")
    bias_d = nc.dram_tensor("bias", [2, 128, 1], dt.float32, kind="ExternalInput")
    wlab_d = nc.dram_tensor("wlabT", [128, 2, V], dt.bfloat16, kind="ExternalInput")
    se_d = nc.dram_tensor("sumexp", [128, RT], dt.float32, kind="ExternalOutput")
    feat_d = nc.dram_tensor("featsT", [2, 128, SPANS], dt.bfloat16,
                            kind="ExternalOutput")

    with tile.TileContext(nc) as tc, ExitStack() as ctx:
        singles = ctx.enter_context(tc.tile_pool(name="singles", bufs=1))
        wpool = ctx.enter_context(tc.tile_pool(name="wpool", bufs=3))
        pspool = ctx.enter_context(tc.tile_pool(name="ps", bufs=2, space="PSUM"))

        xT_sb = singles.tile([128, KB, SPANS], dt.bfloat16)
        nc.sync.dma_start(out=xT_sb[:], in_=xT_d[:])
        wT_sb = singles.tile([128, KB, D], dt.bfloat16)
        nc.sync.dma_start(out=wT_sb[:], in_=wT_d[:])

        # --- FFN: featsT[h*128+p, n] = tanh(sum_k W[d,k] X[n,k] + b[d]) ---
        feats_sb = []
        for h in range(2):
            bias_sb = singles.tile([128, 1], dt.float32, tag=f"bias{h}")
            nc.sync.dma_start(out=bias_sb[:], in_=bias_d[h])
            ps = pspool.tile([128, BIGCOLS], dt.float32, tag="ps")
            for kb in range(KB):
                nc.tensor.matmul(ps[:, :SPANS],
                                 lhsT=wT_sb[:, kb, h * 128:(h + 1) * 128],
                                 rhs=xT_sb[:, kb, :],
                                 start=(kb == 0), stop=(kb == KB - 1))
            fsb = singles.tile([128, SPANS], dt.bfloat16, tag=f"feat{h}")
            nc.scalar.activation(fsb[:], ps[:, :SPANS], AF.Tanh, bias=bias_sb[:])
            nc.sync.dma_start(out=feat_d[h], in_=fsb[:])
            feats_sb.append(fsb)

        # --- vocab sweep: psum tile = [128 rows, <=2048 vocab cols] ---
        partials = singles.tile([128, RT * NBT], dt.float32)
        for bt, (c0, widths) in enumerate(big_tiles):
            wtot = sum(widths)
            wtile = wpool.tile([128, 2, BIGCOLS], dt.bfloat16, tag="w")
            nc.sync.dma_start(out=wtile[:, :, :wtot],
                              in_=wlab_d[:, :, c0:c0 + wtot])
            for rt in range(RT):
                ps = pspool.tile([128, BIGCOLS], dt.float32, tag="ps")
                off = 0
                for w in widths:
                    for kh in range(2):
                        nc.tensor.matmul(ps[:, off:off + w],
                                         lhsT=feats_sb[kh][:, rt * 128:(rt + 1) * 128],
                                         rhs=wtile[:, kh, off:off + w],
                                         start=(kh == 0), stop=(kh == 1))
                    off += w
                q = rt * NBT + bt
                nc.scalar.activation(ps[:, :wtot], ps[:, :wtot], AF.Exp,
                                     accum_out=partials[:, q:q + 1])

        # --- fold the per-big-tile partial sums ---
        se_sb = singles.tile([128, RT], dt.float32)
        dummy = singles.tile([128, NBT], dt.float32)
        for rt in range(RT):
            nc.scalar.activation(dummy[:], partials[:, rt * NBT:(rt + 1) * NBT],
                                 AF.Identity, accum_out=se_sb[:, rt:rt + 1])
        nc.sync.dma_start(out=se_d[:], in_=se_sb[:])

    if split_waits:
        _split_multi_waits(nc, mybir)
    return nc


def _get_program():
    if "nc" not in _CACHE:
        _CACHE["nc"] = _build_program()
    return _CACHE["nc"]


def _pack_core(X, WinT_pad, bias):
    # X: [SPANS, K<=KPAD] f32 span features; returns device arrays
    K = X.shape[1]
    XT = np.zeros((KPAD, SPANS), dtype=BF16)
    XT[:K] = X.T.astype(BF16)
    xT = np.ascontiguousarray(XT.reshape(KB, 128, SPANS).transpose(1, 0, 2))
    bias2 = np.ascontiguousarray(bias.astype(np.float32).reshape(2, 128, 1))
    return {"xT": xT, "wT": WinT_pad, "bias": bias2}


def _prepare(inputs):
    forwards = np.asarray(inputs["forwards"], dtype=np.float32)
    backwards = np.asarray(inputs["backwards"], dtype=np.float32)
    begins = np.asarray(inputs["begins"])
    ends = np.asarray(inputs["ends"])
    bids = np.asarray(inputs["bids"])
    length_emb = np.asarray(inputs["length_emb"], dtype=np.float32)
    W_ctx = np.asarray(inputs["W_ctx"], dtype=np.float32)
    b_ctx = np.asarray(inputs["b_ctx"], dtype=np.float32)
    W_phr = np.asarray(inputs["W_phr"], dtype=np.float32)
    b_phr = np.asarray(inputs["b_phr"], dtype=np.float32)
    W_lab = np.asarray(inputs["W_lab"], dtype=np.float32)

    f_b = forwards[begins - 1, bids]
    f_e = forwards[ends - 1, bids]
    b_e = backwards[ends, bids]
    b_b = backwards[begins, bids]
    lengths = np.minimum(ends - begins, LMAX) - 1
    le = length_emb[lengths]

    ctx_X = np.concatenate([le, f_b, b_e], axis=1)            # [N, 1056]
    phr_X = np.concatenate([le, f_b, f_e, b_e, b_b], axis=1)  # [N, 2080]

    def padWT(Wm):
        WT = np.zeros((KPAD, D), dtype=BF16)
        WT[:Wm.shape[1]] = Wm.T.astype(BF16)
        return np.ascontiguousarray(WT.reshape(KB, 128, D).transpose(1, 0, 2))

    WctxT = padWT(W_ctx)
    WphrT = padWT(W_phr)

    WlabT = W_lab.T.astype(BF16)                               # [D, V]
    wlabT = np.ascontiguousarray(WlabT.reshape(2, 128, V).transpose(1, 0, 2))

    in_maps = []
    for c in range(NCORES):
        if c < 4:
            rows = slice(c * SPANS, (c + 1) * SPANS)
            m = _pack_core(ctx_X[rows], WctxT, b_ctx)
        else:
            rows = slice((c - 4) * SPANS, (c - 3) * SPANS)
            m = _pack_core(phr_X[rows], WphrT, b_phr)
        m["wlabT"] = wlabT
        in_maps.append(m)
    return in_maps


def _postprocess(results, inputs):
    tags = np.asarray(inputs["tags"])
    W_lab = np.asarray(inputs["W_lab"], dtype=np.float32)
    b_lab = np.asarray(inputs["b_lab"], dtype=np.float32)

    sumexp = np.empty((2 * N,), dtype=np.float32)
    feats = np.empty((2 * N, D), dtype=np.float32)
    for c in range(NCORES):
        se = np.asarray(results[c]["sumexp"], dtype=np.float32)     # [128, RT]
        ft = np.asarray(results[c]["featsT"]).astype(np.float32)    # [2,128,SPANS]
        r0 = c * SPANS
        sumexp[r0:r0 + SPANS] = se.T.reshape(SPANS)
        feats[r0:r0 + SPANS] = ft.transpose(2, 0, 1).reshape(SPANS, D)

    tags2 = np.concatenate([tags, tags])
    Wtag = W_lab[tags2].astype(BF16).astype(np.float32)              # [2N, D]
    t = np.einsum("nd,nd->n", feats, Wtag) + b_lab[tags2]
    lse = np.log(sumexp)
    lp = (t - lse).astype(np.float32)
    p = np.exp(lp)
    focal = -(1.0 - p) ** GAMMA * lp
    return np.float32(focal.sum(dtype=np.float64) / (2 * N + 1e-5))


def _numpy_reference(inputs):
    # Exact fallback (handles e.g. nonzero b_lab, which the device path folds
    # only into the tag logit, not the normalizer).
    forwards = np.asarray(inputs["forwards"], dtype=np.float32)
    backwards = np.asarray(inputs["backwards"], dtype=np.float32)
    begins = np.asarray(inputs["begins"])
    ends = np.asarray(inputs["ends"])
    bids = np.asarray(inputs["bids"])
    tags = np.asarray(inputs["tags"])
    length_emb = np.asarray(inputs["length_emb"], dtype=np.float32)
    W_ctx = np.asarray(inputs["W_ctx"], dtype=np.float32)
    b_ctx = np.asarray(inputs["b_ctx"], dtype=np.float32)
    W_phr = np.asarray(inputs["W_phr"], dtype=np.float32)
    b_phr = np.asarray(inputs["b_phr"], dtype=np.float32)
    W_lab = np.asarray(inputs["W_lab"], dtype=np.float32)
    b_lab = np.asarray(inputs["b_lab"], dtype=np.float32)

    f_b = forwards[begins - 1, bids]
    f_e = forwards[ends - 1, bids]
    b_e = backwards[ends, bids]
    b_b = backwards[begins, bids]
    lengths = np.minimum(ends - begins, LMAX) - 1
    le = length_emb[lengths]
    ctx_feat = np.tanh(np.concatenate([le, f_b, b_e], 1) @ W_ctx.T + b_ctx)
    phr_feat = np.tanh(np.concatenate([le, f_b, f_e, b_e, b_b], 1) @ W_phr.T + b_phr)
    feats = np.concatenate([ctx_feat, phr_feat], 0)
    logits = feats @ W_lab.T + b_lab
    m = logits.max(axis=1, keepdims=True)
    lse = (np.log(np.exp(logits - m).sum(axis=1, keepdims=True)) + m)[:, 0]
    tags2 = np.concatenate([tags, tags])
    t = np.take_along_axis(logits, tags2[:, None], axis=1)[:, 0]
    lp = t - lse
    p = np.exp(lp)
    focal = -(1.0 - p) ** GAMMA * lp
    return np.float32(focal.sum() / (2 * N + 1e-5))


def _shapes_ok(inputs):
    try:
        checks = [
            np.shape(inputs["forwards"]) == (S, B, H),
            np.shape(inputs["backwards"]) == (S, B, H),
            np.shape(inputs["begins"]) == (N,),
            np.shape(inputs["W_ctx"]) == (D, 2 * H + LDIM),
            np.shape(inputs["W_phr"]) == (D, 4 * H + LDIM),
            np.shape(inputs["W_lab"]) == (V, D),
            not np.any(np.asarray(inputs["b_lab"])),
        ]
        return all(checks)
    except Exception:
        return False


def run_device(inputs, trace=False):
    """Run the device portion; returns (results, BassKernelResults)."""
    from concourse.bass_utils import run_bass_kernel_spmd
    nc = _get_program()
    in_maps = _prepare(inputs)
    br = run_bass_kernel_spmd(nc, in_maps, list(range(NCORES)), trace=trace)
    return br


def kernel(**inputs):
    if not _shapes_ok(inputs):
        return _numpy_reference(inputs)
    br = run_device(inputs)
    return _postprocess(br.results, inputs)
